# revision 3
# baseline (speedup 1.0000x reference)
"""AdaptiveMixing Trainium2 kernel (8 NeuronCores, pure data parallel).

Math: out[b,s] = sum_k softmax(ada_mask[b,s])[k] * xpad[b, s+k-10]  (K=21)

Key idea: with S=128 on SBUF partitions and H*W on the free dim, the
spectral sliding-window reduction is a single 128x128 banded matmul:
    out[s_o, f] = sum_{s} Wb[s_o, s] * x[s, f],
    Wb[s_o, s] = w[s_o, s - s_o + 10] for |s - s_o| <= 10 else 0
so the TensorEngine does all cross-partition movement:
    out = lhsT.T @ x with lhsT[s, s_o] = Wb[s_o, s].

lhsT is built on-device from the softmaxed mask via a diagonal-scatter
DMA through a small DRAM scratch: element (s_o, k) of the (128,21)
weight tile lands at flat offset s_o*129 + k*128 of a (148,128) scratch,
i.e. scratch[s_o + k, s_o]; rows 10..137 read back as lhsT.

Sharding (host side): core i <- batch b = i//2, H-half h = i%2.
Each core handles x[b, :, h*64:(h+1)*64, :] as a (128, 8192) slab.
No communication needed.
"""

import numpy as np

B, S, H, W = 4, 128, 128, 128
K = 21
PAD = 10
N_CORES = 8
H_SPLIT = 2
HS = H // H_SPLIT          # 64 rows of H per core
FREE = HS * W              # 8192
CHUNK = 1024               # free-dim elements per DMA chunk
MM_N = 512                 # matmul free dim per instruction

_COMPILED = {}


def _build_nc():
    import concourse.bass as bass
    import concourse.mybir as mybir
    import concourse.tile as tile
    from concourse import bacc

    f32 = mybir.dt.float32
    # Bacc (not Bass): its compile() legalizes sem waits to <=1 per
    # instruction, which this walrus requires.
    nc = bacc.Bacc()
    x_d = nc.declare_dram_parameter("x", [S, FREE], f32, isOutput=False)
    m_d = nc.declare_dram_parameter("mask", [S, K], f32, isOutput=False)
    o_d = nc.declare_dram_parameter("out", [S, FREE], f32, isOutput=True)

    SCR_ROWS = S + 2 * PAD  # 148

    with tile.TileContext(nc) as tc:
        with (
            tc.tile_pool(name="singles", bufs=1) as singles,
            tc.tile_pool(name="xin", bufs=3) as xin,
            tc.tile_pool(name="oout", bufs=3) as oout,
            tc.tile_pool(name="psum", bufs=4, space="PSUM") as psum,
            tc.tile_pool(name="dram", bufs=1, space="DRAM") as dram,
        ):
            # ---- softmax over the 21-tap window (per partition = band s) ----
            mask_t = singles.tile([S, K], f32)
            nc.sync.dma_start(out=mask_t[:], in_=m_d[:])

            mx = singles.tile([S, 1], f32)
            nc.vector.reduce_max(mx[:], mask_t[:], axis=mybir.AxisListType.X)
            negmx = singles.tile([S, 1], f32)
            nc.vector.tensor_scalar_mul(negmx[:], mx[:], -1.0)

            wexp = singles.tile([S, K], f32)
            wsum = singles.tile([S, 1], f32)
            nc.scalar.activation(
                out=wexp[:],
                in_=mask_t[:],
                func=mybir.ActivationFunctionType.Exp,
                bias=negmx[:],
                scale=1.0,
                accum_out=wsum[:],
            )
            rsum = singles.tile([S, 1], f32)
            nc.vector.reciprocal(rsum[:], wsum[:])
            wn = singles.tile([S, K], f32)
            nc.vector.tensor_scalar_mul(wn[:], wexp[:], rsum[:])

            # ---- scatter softmaxed taps into the banded lhsT via DRAM ----
            scratch = dram.tile([SCR_ROWS * S], f32)
            zview = scratch.rearrange("(p f) -> p f", p=S)  # (128, 148)
            ztile = singles.tile([S, SCR_ROWS], f32)
            nc.vector.memset(ztile[:], 0.0)
            nc.sync.dma_start(out=zview, in_=ztile[:])

            diag_ap = bass.AP(
                tensor=scratch.tensor,
                offset=scratch.offset,
                ap=[[S + 1, S], [S, K]],  # elem (s,k) -> s*129 + k*128
            )
            with nc.allow_non_contiguous_dma(reason="banded diagonal scatter"):
                nc.sync.dma_start(out=diag_ap, in_=wn[:])

            band_view = scratch.rearrange("(r c) -> r c", r=SCR_ROWS)[PAD : PAD + S, :]
            band = singles.tile([S, S], f32)
            nc.sync.dma_start(out=band[:], in_=band_view)

            # ---- stream x through the banded matmul ----
            n_chunks = FREE // CHUNK
            mm_per_chunk = CHUNK // MM_N
            for c in range(n_chunks):
                xt = xin.tile([S, CHUNK], f32)
                nc.sync.dma_start(
                    out=xt[:], in_=x_d[:, c * CHUNK : (c + 1) * CHUNK]
                )
                ot = oout.tile([S, CHUNK], f32)
                for j in range(mm_per_chunk):
                    ps = psum.tile([S, MM_N], f32)
                    nc.tensor.matmul(
                        ps[:],
                        lhsT=band[:],
                        rhs=xt[:, j * MM_N : (j + 1) * MM_N],
                        start=True,
                        stop=True,
                    )
                    nc.vector.tensor_copy(
                        out=ot[:, j * MM_N : (j + 1) * MM_N], in_=ps[:]
                    )
                nc.sync.dma_start(
                    out=o_d[:, c * CHUNK : (c + 1) * CHUNK], in_=ot[:]
                )

    nc.finalize()
    return nc


def _get_compiled():
    if "nc" not in _COMPILED:
        _COMPILED["nc"] = _build_nc()
    return _COMPILED["nc"]


def _shard_inputs(x, ada_mask):
    in_maps = []
    for i in range(N_CORES):
        b, h = divmod(i, H_SPLIT)
        xs = np.ascontiguousarray(
            x[b, :, h * HS : (h + 1) * HS, :].reshape(S, FREE)
        ).astype(np.float32, copy=False)
        ms = np.ascontiguousarray(ada_mask[b]).astype(np.float32, copy=False)
        in_maps.append({"x": xs, "mask": ms})
    return in_maps


def _run(x, ada_mask, trace=False, tmpdir=None):
    from concourse.bass_utils import run_bass_kernel_spmd

    nc = _get_compiled()
    in_maps = _shard_inputs(x, ada_mask)
    res = run_bass_kernel_spmd(
        nc,
        in_maps,
        core_ids=list(range(N_CORES)),
        trace=trace,
        tmpdir=tmpdir,
    )
    out = np.empty((B, S, H, W), dtype=np.float32)
    for i in range(N_CORES):
        b, h = divmod(i, H_SPLIT)
        out[b, :, h * HS : (h + 1) * HS, :] = res.results[i]["out"].reshape(S, HS, W)
    return out, res


def kernel(x, ada_mask):
    x = np.asarray(x)
    ada_mask = np.asarray(ada_mask)
    out, _ = _run(x, ada_mask, trace=False)
    return out


def kernel_traced(x, ada_mask, tmpdir=None):
    """Correctness + profile run: returns (out, BassKernelResults)."""
    x = np.asarray(x)
    ada_mask = np.asarray(ada_mask)
    return _run(np.asarray(x), np.asarray(ada_mask), trace=True, tmpdir=tmpdir)


# revision 6
# speedup vs baseline: 1.5413x; 1.5413x over previous
"""AdaptiveMixing Trainium2 kernel (8 NeuronCores, pure data parallel).

Math: out[b,s] = sum_k softmax(ada_mask[b,s])[k] * xpad[b, s+k-10]  (K=21)

Key idea: with S=128 on SBUF partitions and H*W on the free dim, the
spectral sliding-window reduction is a single 128x128 banded matmul
per free-dim tile:
    out[s_o, f] = sum_{s} Wb[s_o, s] * x[s, f],
    Wb[s_o, s] = w[s_o, s - s_o + 10] for |s - s_o| <= 10 else 0
so the TensorEngine does all cross-partition movement:
    out = lhsT.T @ x with lhsT[s, s_o] = Wb[s_o, s].

Band build (on device, off the DMA path):
  1. dstack[p,k,f] = 1 if f == p + k - 10 else 0   (gpsimd affine_select,
     no input dependency -- runs at kernel start)
  2. softmax numerator wexp = exp(mask - max) (+ row sums via accum_out);
     normalization is folded into the PSUM->SBUF copies as a per-partition
     reciprocal multiply.
  3. DW = dstack * wexp (broadcast along f)        (one DVE op)
  4. E'[p,f] = sum_k DW[p,k,f]                     (DVE reduce over k)
     E'[s_o, s_src] = wexp[s_o, s_src - s_o + 10]
  5. band = E'.T via PE transpose                  (lhsT for the big matmuls)

Sharding (host side): core i <- batch b = i//2, H-half h = i%2.
Each core handles x[b, :, h*64:(h+1)*64, :] as a (128, 8192) slab.
No communication needed.
"""

import os

import numpy as np

B, S, H, W = 4, 128, 128, 128
K = 21
PAD = 10
N_CORES = 8
H_SPLIT = 2
HS = H // H_SPLIT          # 64 rows of H per core
FREE = HS * W              # 8192
CHUNK = 2048               # free-dim elements per DMA chunk
MM_N = 512                 # matmul free dim per instruction

# fp32 PE matmul runs at 4 cycles/col; float32r (same 4-byte data) runs at
# 1 cycle/col for free >= 256. Toggle for A/B testing.
USE_F32R = os.environ.get("KERNEL_F32", "") != "1"

_COMPILED = {}


def _build_nc():
    import concourse.bass as bass
    import concourse.mybir as mybir
    import concourse.tile as tile
    from concourse import bacc

    f32 = mybir.dt.float32
    mm_dt = mybir.dt.float32r if USE_F32R else f32
    # Bacc (not Bass): its compile() legalizes sem waits to <=1 per
    # instruction, which this walrus requires.
    nc = bacc.Bacc()
    x_d = nc.declare_dram_parameter("x", [S, FREE], mm_dt, isOutput=False)
    m_d = nc.declare_dram_parameter("mask", [S, K], f32, isOutput=False)
    o_d = nc.declare_dram_parameter("out", [S, FREE], f32, isOutput=True)

    with tile.TileContext(nc) as tc:
        with (
            tc.tile_pool(name="singles", bufs=1) as singles,
            tc.tile_pool(name="xin", bufs=4) as xin,
            tc.tile_pool(name="oout", bufs=4) as oout,
            tc.tile_pool(name="psum", bufs=6, space="PSUM") as psum,
            tc.tile_pool(name="psumT", bufs=1, space="PSUM") as psumT,
        ):
            # ---- shifted-identity stack: no input deps, starts immediately
            dstack = singles.tile([S, K, S], f32)
            nc.gpsimd.memset(dstack[:], 0.0)
            nc.gpsimd.affine_select(
                out=dstack[:],
                in_=dstack[:],
                compare_op=mybir.AluOpType.not_equal,
                fill=1.0,
                base=-PAD,
                # affine(p,k,f) = p + k - f - PAD ; == 0 -> fill 1.0
                pattern=[[1, K], [-1, S]],
                channel_multiplier=1,
            )

            # ---- softmax numerator (normalization folded into epilogue) ----
            mask_t = singles.tile([S, K], f32)
            nc.sync.dma_start(out=mask_t[:], in_=m_d[:])

            mx = singles.tile([S, 1], f32)
            nc.vector.reduce_max(mx[:], mask_t[:], axis=mybir.AxisListType.X)
            negmx = singles.tile([S, 1], f32)
            nc.vector.tensor_scalar_mul(negmx[:], mx[:], -1.0)

            wexp = singles.tile([S, K], f32)
            wsum = singles.tile([S, 1], f32)
            nc.scalar.activation(
                out=wexp[:],
                in_=mask_t[:],
                func=mybir.ActivationFunctionType.Exp,
                bias=negmx[:],
                scale=1.0,
                accum_out=wsum[:],
            )
            rsum = singles.tile([S, 1], f32)
            nc.vector.reciprocal(rsum[:], wsum[:])

            # ---- banded weight matrix ----
            dw = singles.tile([S, K, S], f32)
            nc.vector.tensor_tensor(
                dw[:],
                dstack[:],
                wexp[:, :, None].to_broadcast((S, K, S)),
                mybir.AluOpType.mult,
            )
            eprime = singles.tile([S, S], f32)
            nc.vector.reduce_sum(
                eprime[:],
                dw[:].rearrange("p k f -> p f k"),
                axis=mybir.AxisListType.X,
            )
            band_ps = psumT.tile([S, S], f32)
            nc.tensor.transpose(band_ps[:], eprime[:], dstack[:, PAD, :])
            band = singles.tile([S, S], mm_dt)
            nc.vector.tensor_copy(out=band[:], in_=band_ps[:])

            # ---- stream x through the banded matmul ----
            n_chunks = FREE // CHUNK
            mm_per_chunk = CHUNK // MM_N
            for c in range(n_chunks):
                xt = xin.tile([S, CHUNK], mm_dt)
                nc.sync.dma_start(
                    out=xt[:], in_=x_d[:, c * CHUNK : (c + 1) * CHUNK]
                )
                ot = oout.tile([S, CHUNK], f32)
                for j in range(mm_per_chunk):
                    ps = psum.tile([S, MM_N], f32)
                    nc.tensor.matmul(
                        ps[:],
                        lhsT=band[:],
                        rhs=xt[:, j * MM_N : (j + 1) * MM_N],
                        start=True,
                        stop=True,
                    )
                    # epilogue: copy + softmax denominator (per-partition)
                    nc.vector.tensor_scalar_mul(
                        ot[:, j * MM_N : (j + 1) * MM_N], ps[:], rsum[:]
                    )
                nc.sync.dma_start(
                    out=o_d[:, c * CHUNK : (c + 1) * CHUNK], in_=ot[:]
                )

    nc.finalize()
    return nc


def _get_compiled():
    if "nc" not in _COMPILED:
        _COMPILED["nc"] = _build_nc()
    return _COMPILED["nc"]


def _shard_inputs(x, ada_mask):
    in_maps = []
    for i in range(N_CORES):
        b, h = divmod(i, H_SPLIT)
        xs = np.ascontiguousarray(
            x[b, :, h * HS : (h + 1) * HS, :].reshape(S, FREE)
        ).astype(np.float32, copy=False)
        ms = np.ascontiguousarray(ada_mask[b]).astype(np.float32, copy=False)
        in_maps.append({"x": xs, "mask": ms})
    return in_maps


def _run(x, ada_mask, trace=False, tmpdir=None):
    from concourse.bass_utils import run_bass_kernel_spmd

    nc = _get_compiled()
    in_maps = _shard_inputs(x, ada_mask)
    res = run_bass_kernel_spmd(
        nc,
        in_maps,
        core_ids=list(range(N_CORES)),
        trace=trace,
        tmpdir=tmpdir,
    )
    out = np.empty((B, S, H, W), dtype=np.float32)
    for i in range(N_CORES):
        b, h = divmod(i, H_SPLIT)
        out[b, :, h * HS : (h + 1) * HS, :] = res.results[i]["out"].reshape(S, HS, W)
    return out, res


def kernel(x, ada_mask):
    x = np.asarray(x)
    ada_mask = np.asarray(ada_mask)
    out, _ = _run(x, ada_mask, trace=False)
    return out


def kernel_traced(x, ada_mask, tmpdir=None):
    """Correctness + profile run: returns (out, BassKernelResults)."""
    return _run(np.asarray(x), np.asarray(ada_mask), trace=True, tmpdir=tmpdir)
